# revision 72
# baseline (speedup 1.0000x reference)
"""Trainium2 Bass kernel for nn_FLAttention (sparse_attention).

Math (per batch b, head h), with q = aq*x+bq, k = ak*x+bk, v = av*x+bv:
  S[i,j] = 1/(|k_j - q_i| + eps);  P = softmax_j(S);  att_i = sum_j P_ij v_j / sqrt(H)
  out = x + sum_h att

Pipeline per (b,h) pair and 128-query i-tile (D=1024 -> 8 tiles):
  PE  : dp[i,j] = ak*x_j - aq*x_i + (bk-bq) + eps via one K=3 matmul per
        512-chunk -> PSUM (f32r operands, ~11-bit mantissa).
  DVE : custom fused op RECIP_ABSMAX_ANT reads dp from PSUM and computes
        r = recip1NR(max(dp, 2eps-dp)) = ~1/(|d|+eps) (0.4% approx, fp32),
        with a free row-max accumulate m (bit-consistent with r: the exp
        below peaks at exactly 1 and Z >= 1).
  ACT : p = Exp(r - m), accum_out Z (fp32).
  Pool/DVE (split per tile): px = p * x (tensor_tensor; bf16 out)
  DVE : tensor_scalar(px * avs_h) at 4x bf16 with accum -> ns column.
  Epilogue per pair (pipelined into the next pair): att = ns * (1/Z);
        accumulate over heads; per batch: y = x + sum_h att + sum_h bv/sqrt(H).

All tiny derived constants (ak, -aq, bk-bq+eps, av/sqrt(H), sum_h bv/sqrt(H)),
the f32r pre-rounding of matmul operand sources, and the bf16 copy of x are
computed on the host in kernel() and passed as auxiliary device inputs - the
device prologue is just ~17 DMA descriptors issued ahead of the first matmul.
The value ops and per-pair epilogues are software-pipelined across pairs so
the DVE (the bottleneck engine, ~83% occupancy) never drains.

The custom DVE op is registered at runtime (row 17 of the custom-DVE table);
its 8-stage body is: x = max(Src0, C2 - Src0); bitwise-NOT exponent-flip
seed; one Newton step; max-accumulate.

Sharding: data-parallel over batch: B=16 -> 2 batches per core on 8 cores.
"""
import numpy as np
import ml_dtypes

import concourse.bass as bass
import concourse.bacc as bacc
import concourse.mybir as mybir
import concourse.tile as tile
from concourse.bass_utils import run_bass_kernel_spmd

B, D, H = 16, 1024, 4
N_CORES = 8
BPC = B // N_CORES          # batches per core
NPAIR = BPC * H             # (b,h) pairs per core
NT = D // 128               # i-tiles per pair
EPS = 1e-8
ISH = float(1.0 / np.sqrt(np.float32(H)))  # 1/sqrt(H) = 0.5

F32 = mybir.dt.float32
F32R = mybir.dt.float32r
BF16 = mybir.dt.bfloat16
AX = mybir.AxisListType
OP = mybir.AluOpType
AF = mybir.ActivationFunctionType

F32R_MM = True                      # fp32r matmuls (1 cyc/row vs 4)
DVE_TT_TILES = frozenset({1, 5})    # tiles whose p*x runs on DVE, rest on Pool
VAL_LAG = 6                         # value-op software-pipeline depth

# ---------------- custom DVE op: r = ~1/(|d|+eps) with row-max accum --------
from concourse.dve_spec import (Spec, Src0, C0, C1, C2, Zero, Bin, AluOp,
                                 maxx, lower)
from concourse.dve_uop import DveOpSpec
from concourse.dve_ops import DveOp, RECIP_APPROX_FAST_CONSTS
import concourse.dve_ops as dve_ops

RECIP_NAME = "RECIP_ABSMAX_ANT"
C0V = RECIP_APPROX_FAST_CONSTS["s0"]
C1V = RECIP_APPROX_FAST_CONSTS["s1"]


def _recip_absmax_ref(in0, in1, c0, c1, c2):
    # in0 = d+eps; x = max(in0, c2-in0) = |d|+eps (c2 = 2eps);
    # out = 1-NR approx of 1/x; accum = max over free dim, seeded at 0
    x = np.maximum(in0.astype(np.float32),
                   (np.float32(c2) - in0).astype(np.float32))
    not_x = (~x.view(np.int32)).view(np.float32)
    y0 = not_x * np.float32(c0)
    y1 = (y0 * (np.float32(c1) - x * y0)).astype(np.float32)
    P = y1.shape[0]
    body = y1.reshape(P, -1)
    acc = np.maximum(np.float32(0.0), body.max(axis=-1, keepdims=True))
    return body, acc


def _register_recip_op():
    if RECIP_NAME in dve_ops._SUB_OPCODE_FOR_NAME:
        for o in dve_ops.OPS:
            if o.name == RECIP_NAME:
                return o
    x = Bin(AluOp.MAX, Src0, Bin(AluOp.SUBTRACT, C2, Src0))
    nx = Bin(AluOp.BITWISE_NOT, x, x)
    y0 = Bin(AluOp.MULTIPLY, nx, C0)
    t = Bin(AluOp.MULTIPLY, x, y0)
    y1 = Bin(AluOp.MULTIPLY, y0, Bin(AluOp.SUBTRACT, C1, t))
    spec = Spec(body=y1, accum=maxx, accum_init=Zero, reference=_recip_absmax_ref)
    row = max(dve_ops._SUB_OPCODE_FOR_NAME.values()) + 1
    assert row < 0x20
    dve_ops._SUB_OPCODE_FOR_NAME[RECIP_NAME] = row
    shas = {}
    for ver in ("v3", "v4"):
        s = DveOpSpec(name=RECIP_NAME, opcode=row, uops=lower(spec, ver=ver),
                      rd1_en=False)
        shas[ver] = s.sha(ver)
    op = DveOp(RECIP_NAME, spec, subdim=False, uops_sha=shas)
    dve_ops.OPS.append(op)
    dve_ops.CUSTOM_DVE_SPECS[RECIP_NAME] = spec
    return op


RECIP_OP = _register_recip_op()


def build_bass():
    nc = bacc.Bacc(
        "TRN2",
        target_bir_lowering=False,
        debug=False,
        enable_asserts=False,
        num_devices=N_CORES,
    )
    x_d = nc.dram_tensor("x", (BPC, D), F32, kind="ExternalInput").ap()
    # host-derived aux inputs (see kernel()):
    # cst = [ak(H); -aq(H); bk-bq+eps(H)] (f32r-pre-rounded bits when F32R_MM)
    cst_d = nc.dram_tensor("cst", (1, 3 * H), F32, kind="ExternalInput").ap()
    # xr = [rounded x rows (BPC); ones row]
    xr_d = nc.dram_tensor("xr", (BPC + 1, D), F32, kind="ExternalInput").ap()
    xbf_d = nc.dram_tensor("xbf", (BPC, D), BF16, kind="ExternalInput").ap()
    avs_d = nc.dram_tensor("avsr", (1, H), F32, kind="ExternalInput").ap()
    bvs_d = nc.dram_tensor("bvsr", (1, 1), F32, kind="ExternalInput").ap()
    y_d = nc.dram_tensor("y", (BPC, D), F32, kind="ExternalOutput").ap()

    x_col_v = x_d.rearrange("b (t p) -> b p t", p=128)
    y_col_v = y_d.rearrange("b (t p) -> b p t", p=128)

    MMD = F32R if F32R_MM else F32

    def mmview(ap):
        return ap.bitcast(F32R) if F32R_MM else ap

    def bcast_part(src: bass.AP, n_part: int):
        # replicate a (1, n) DRAM row across n_part partitions
        return bass.AP(tensor=src.tensor, offset=src.offset,
                       ap=[[0, n_part]] + list(src.ap[1:]))

    def refree(sl: bass.AP, free_ap):
        # keep a slice's partition entry + offset, replace free dims
        return bass.AP(tensor=sl.tensor, offset=sl.offset,
                       ap=[list(sl.ap[0])] + free_ap)

    with tile.TileContext(nc) as tc:
        with (
            tc.tile_pool(name="singles", bufs=1) as singles,
            tc.tile_pool(name="psum", bufs=3, space="PSUM") as psum,
            tc.tile_pool(name="psumw", bufs=1, space="PSUM") as psumw,
            tc.tile_pool(name="bigr", bufs=4) as bigr,       # r tiles
            tc.tile_pool(name="bigp", bufs=8) as bigp,       # p tiles
            tc.tile_pool(name="bigpx", bufs=5) as bigpx,     # px tiles (bf16)
            tc.tile_pool(name="bigs", bufs=3) as bigs,       # ts-val discard
            tc.tile_pool(name="smalls", bufs=6) as smalls,
        ):
            # ---------- one-time prep: pure DMA, pair-0 path first ----------
            # operands: one big tile per side, h-major columns (q = h*BPC + b);
            # lhsT rows = [ak; x; 1], rhs rows = [x; -aq; cce]
            big_lhsT = singles.tile([3, NPAIR * D], MMD, tag="big_lhsT")
            big_rhs = singles.tile([3, NPAIR * D], MMD, tag="big_rhs")

            def cst_bcast(idx, n):
                src = cst_d[0:1, idx:idx + 1]
                ap = bass.AP(tensor=src.tensor, offset=src.offset,
                             ap=[[0, 1], [0, n], [1, 1]])
                return mmview(ap)

            def xr_rep(row, nrep):
                src = xr_d[row:row + 1, :]
                ap = bass.AP(tensor=src.tensor, offset=src.offset,
                             ap=[[0, 1], [0, nrep], [1, D]])
                return mmview(ap)

            # head-0 consts + batch-0-relevant rows first, then the rest
            for h in range(H):
                cs = slice(h * BPC * D, (h + 1) * BPC * D)
                nc.sync.dma_start(out=big_lhsT[0:1, cs],
                                  in_=cst_bcast(h, BPC * D))
                nc.sync.dma_start(out=big_rhs[1:2, cs],
                                  in_=cst_bcast(H + h, BPC * D))
                nc.sync.dma_start(out=big_rhs[2:3, cs],
                                  in_=cst_bcast(2 * H + h, BPC * D))
                if h == 0:
                    # batch-0 rows + ones first (pair 0's critical path),
                    # batch-1 rows after
                    nc.scalar.dma_start(
                        out=refree(big_lhsT[1:2, 0:1], [[BPC * D, H], [1, D]]),
                        in_=xr_rep(0, H))
                    nc.scalar.dma_start(
                        out=refree(big_rhs[0:1, 0:1], [[BPC * D, H], [1, D]]),
                        in_=xr_rep(0, H))
                    nc.scalar.dma_start(out=big_lhsT[2:3, :],
                                        in_=xr_rep(BPC, NPAIR))
                    for b in range(1, BPC):
                        nc.scalar.dma_start(
                            out=refree(big_lhsT[1:2, b * D:b * D + 1],
                                       [[BPC * D, H], [1, D]]),
                            in_=xr_rep(b, H))
                        nc.scalar.dma_start(
                            out=refree(big_rhs[0:1, b * D:b * D + 1],
                                       [[BPC * D, H], [1, D]]),
                            in_=xr_rep(b, H))

            def opcol(p):
                b, h = divmod(p, H)
                q = h * BPC + b
                return slice(q * D, (q + 1) * D)

            lhsT3 = [big_lhsT[0:3, opcol(p)] for p in range(NPAIR)]
            rhs3 = [big_rhs[0:3, opcol(p)] for p in range(NPAIR)]

            # x broadcast (fp32 + bf16), x column layout, value params
            x_bcast = []
            xbh = []
            x_col = []
            for b in range(BPC):
                xb = singles.tile([128, D], F32, tag=f"x_bcast{b}")
                nc.scalar.dma_start(
                    out=xb,
                    in_=bass.AP(tensor=x_d.tensor, offset=x_d.offset + b * D,
                                ap=[[0, 128], [1, D]]),
                )
                x_bcast.append(xb)
                xh = singles.tile([128, D], BF16, tag=f"xbh{b}")
                nc.gpsimd.tensor_copy(out=xh, in_=xb)
                xbh.append(xh)
                xc = singles.tile([128, NT], F32, tag=f"x_col{b}")
                nc.sync.dma_start(out=xc, in_=x_col_v[b])
                x_col.append(xc)

            av128 = singles.tile([128, H], F32, tag="av128")
            nc.sync.dma_start(out=av128, in_=bcast_part(avs_d, 128))
            avs = singles.tile([128, H], F32, tag="avs")
            nc.vector.tensor_scalar(out=avs, in0=av128, scalar1=1.0,
                                    scalar2=None, op0=OP.mult)
            bv128 = singles.tile([128, 1], F32, tag="bv128")
            nc.sync.dma_start(out=bv128, in_=bcast_part(bvs_d, 128))
            bvsum = singles.tile([128, 1], F32, tag="bvsum")
            nc.vector.tensor_scalar(out=bvsum, in0=bv128, scalar1=1.0,
                                    scalar2=None, op0=OP.mult)

            # PE p-state warmup: dummy matmuls on never-written scratch
            # (no deps, discarded output) keep the PE continuously busy from
            # t~0 so the first real matmuls run at full clock
            warm_in = singles.tile([3, 128], F32, tag="warm_in")
            nc.gpsimd.memset(warm_in, 1.0)
            warm_out = psumw.tile([128, 128], F32, tag="warm")
            for _ in range(8):
                nc.tensor.matmul(warm_out, warm_in[0:3, 0:128],
                                 warm_in[0:3, 0:128], start=True, stop=True)

            # ---------- main loops ----------
            acc_of = {}          # b -> running head accumulator tile
            pending_val = []     # [(b, h, t, p_t, ns8)]
            pending_epi = []     # [(b, h, z8, ns8)]

            def dve_tiles_of(vb, vh):
                # last pair runs more value-multiplies on the DVE so the Pool
                # tail drain after the final exp stays short; first two pairs
                # likewise so Pool's pipeline-fill backlog stays small
                if vb == BPC - 1 and vh == H - 1:
                    return frozenset({1, 3, 5, 7})
                if vb == 0 and vh == 0:
                    return frozenset({1, 3, 5, 7})
                if vb == 0 and vh == 1:
                    return frozenset({1, 3, 5})
                return DVE_TT_TILES

            def do_val(vb, vh, t, p_t, ns8):
                px_t = bigpx.tile([128, D], BF16, tag="px")
                if t in dve_tiles_of(vb, vh):
                    nc.vector.tensor_tensor(out=px_t, in0=p_t,
                                            in1=xbh[vb], op=OP.mult)
                else:
                    nc.gpsimd.tensor_tensor(out=px_t, in0=p_t,
                                            in1=x_bcast[vb], op=OP.mult)
                s_t = bigs.tile([128, D], BF16, tag="s")
                nc.vector.tensor_scalar(
                    out=s_t, in0=px_t, scalar1=avs[:, vh:vh + 1],
                    scalar2=0.0, op0=OP.mult, op1=OP.add,
                    accum_out=ns8[:, t:t + 1],
                )

            def do_epi(eb, eh, z8, ns8):
                # att_h = ns / Z ; acc += att_h; after the last head: y out
                rz8 = smalls.tile([128, NT], F32, tag="rz8")
                nc.vector.reciprocal(out=rz8, in_=z8)
                acc = acc_of.get(eb)
                acc_new = smalls.tile([128, NT], F32, tag=f"acc{eb}_{eh}")
                if acc is None:
                    nc.vector.tensor_tensor(out=acc_new, in0=ns8, in1=rz8,
                                            op=OP.mult)
                else:
                    t2 = smalls.tile([128, NT], F32, tag="t2")
                    nc.vector.tensor_tensor(out=t2, in0=ns8, in1=rz8,
                                            op=OP.mult)
                    nc.gpsimd.tensor_tensor(out=acc_new, in0=acc, in1=t2,
                                            op=OP.add)
                acc_of[eb] = acc_new
                if eh == H - 1:
                    yb8 = smalls.tile([128, NT], F32, tag="yb8")
                    nc.scalar.activation(out=yb8, in_=acc_new, func=AF.Identity,
                                         bias=bvsum, scale=1.0)
                    y8 = smalls.tile([128, NT], F32, tag="y8")
                    nc.vector.tensor_tensor(out=y8, in0=yb8, in1=x_col[eb],
                                            op=OP.add)
                    nc.sync.dma_start(out=y_col_v[eb], in_=y8)

            for b in range(BPC):
                for h in range(H):
                    p = b * H + h
                    lt = lhsT3[p]
                    rt = rhs3[p]

                    mt8 = smalls.tile([128, NT], F32, tag="mt8")
                    nm8 = smalls.tile([128, NT], F32, tag="nm8")
                    z8 = smalls.tile([128, NT], F32, tag="z8")
                    ns8 = smalls.tile([128, NT], F32, tag="ns8")

                    for t in range(NT):
                        d2 = psum.tile([128, D], F32, tag="d2")
                        lt_sl = lt[0:3, t * 128:(t + 1) * 128]
                        for c in range(2):
                            js = slice(c * 512, (c + 1) * 512)
                            nc.tensor.matmul(d2[:, js], lt_sl, rt[0:3, js],
                                             start=True, stop=True)
                        # fused: r = ~1/(|d|+eps) fp32, m = row max (fp32)
                        r_t = bigr.tile([128, D], F32, tag="r")
                        nc.vector._custom_dve(RECIP_OP, out=r_t, in0=d2,
                                              s0=C0V, s1=C1V, imm2=2.0 * EPS,
                                              accum_out=mt8[:, t:t + 1])
                        # bias = -m on ACT (slack engine)
                        nc.scalar.activation(out=nm8[:, t:t + 1],
                                             in_=mt8[:, t:t + 1],
                                             func=AF.Copy, scale=-1.0)
                        # p = exp(r - m), Z accum fp32; p dtype matches the
                        # tile's value-path engine (bf16 only where DVE's
                        # 2x tensor_tensor needs 2-byte operands)
                        p_t = bigp.tile([128, D],
                                        BF16 if t in dve_tiles_of(b, h) else F32,
                                        tag="p")
                        nc.scalar.activation(out=p_t, in_=r_t, func=AF.Exp,
                                             bias=nm8[:, t:t + 1], scale=1.0,
                                             accum_out=z8[:, t:t + 1])
                        pending_val.append((b, h, t, p_t, ns8))
                        lag = VAL_LAG if p > 0 else max(2, t - 2)
                        if len(pending_val) > lag:
                            do_val(*pending_val.pop(0))
                        # run the previous pair's epilogue once ALL of its
                        # value ops (ns8 columns) have been emitted
                        if t == VAL_LAG + 1 and pending_epi:
                            eb, eh, _, _ = pending_epi[0]
                            ep = eb * H + eh
                            while pending_val and (pending_val[0][0] * H
                                                   + pending_val[0][1]) <= ep:
                                do_val(*pending_val.pop(0))
                            do_epi(*pending_epi.pop(0))
                    pending_epi.append((b, h, z8, ns8))

            while pending_val:
                do_val(*pending_val.pop(0))
            while pending_epi:
                do_epi(*pending_epi.pop(0))

    nc.compile()
    return nc


_NC_CACHE = {}


def _get_nc():
    if "nc" not in _NC_CACHE:
        _NC_CACHE["nc"] = build_bass()
    return _NC_CACHE["nc"]


def _round_f32r(a):
    # pre-round fp32 bits for f32r matmul operands (13-bit mantissa, RNE-ish)
    a = np.ascontiguousarray(np.asarray(a, np.float32))
    ai = a.view(np.int32)
    out = ((ai + np.int32(1 << 9)) >> 10) << 10
    return out.view(np.float32)


def kernel(**inputs) -> np.ndarray:
    x = np.ascontiguousarray(np.asarray(inputs["x"], dtype=np.float32))
    aq = np.asarray(inputs["alpha_q"], np.float32)
    bq = np.asarray(inputs["beta_q"], np.float32)
    ak = np.asarray(inputs["alpha_k"], np.float32)
    bk = np.asarray(inputs["beta_k"], np.float32)
    av = np.asarray(inputs["alpha_v"], np.float32)
    bv = np.asarray(inputs["beta_v"], np.float32)

    cst = np.concatenate(
        [ak, -aq, (bk - bq + np.float32(EPS))], axis=1).astype(np.float32)
    if F32R_MM:
        cst = _round_f32r(cst)
    avsr = (av * np.float32(ISH)).astype(np.float32)
    bvsr = np.array([[np.float32((bv * ISH).sum())]], dtype=np.float32)

    nc = _get_nc()
    in_maps = []
    for c in range(N_CORES):
        xs = x[c * BPC:(c + 1) * BPC]
        xrows = _round_f32r(xs) if F32R_MM else xs
        xr = np.concatenate([xrows, np.ones((1, D), np.float32)], axis=0)
        m = {
            "x": xs,
            "cst": cst,
            "xr": np.ascontiguousarray(xr),
            "xbf": np.ascontiguousarray(xs.astype(ml_dtypes.bfloat16)),
            "avsr": avsr,
            "bvsr": bvsr,
        }
        in_maps.append(m)
    res = run_bass_kernel_spmd(nc, in_maps, core_ids=list(range(N_CORES)))
    return np.concatenate([r["y"] for r in res.results], axis=0)


if __name__ == "__main__":
    rng = np.random.default_rng(0)
    demo = {
        "x": rng.standard_normal((B, D), dtype=np.float32),
        "alpha_q": rng.random((1, H), dtype=np.float32),
        "beta_q": np.zeros((1, H), np.float32),
        "alpha_k": rng.random((1, H), dtype=np.float32),
        "beta_k": np.zeros((1, H), np.float32),
        "alpha_v": rng.random((1, H), dtype=np.float32),
        "beta_v": np.zeros((1, H), np.float32),
    }
    out = kernel(**demo)
    print("kernel output", out.shape, out.dtype)
